# revision 12
# baseline (speedup 1.0000x reference)
"""CPC loss kernel for Trainium2, batch-sharded across 8 NeuronCores.

Shapes (hardcoded per problem spec):
  z, c: [2048, 64, 128] f32;  mask, neg_map: [128, 64] int;  W: [128, 128] f32
  ln_weight/ln_bias: [128] f32.  Output: scalar f32.

Per-core plan (Bc = 8 batch elements), bf16 data path:
  - Host packs per-core row tables: the pos/neg z gathers (with collided
    negatives zeroed, reproducing mask_from_map) land as zg [128L, 16seg*128]
    bf16, the c gather lands pre-transposed as cgt [128c, 8b*128L] bf16, and
    wpk = [W'^T | I].  Device reads only ~832KB contiguous per core.
  - LN stats via per-segment bn_stats (DVE); even/odd recombine and the
    Quake-rsqrt (2 Newton iters, eps and the /128 folded into the magic
    constants) run on the otherwise-idle gpsimd, so ACT only ever needs the
    exp_and_others table set (one load, prefetched by a dummy at t~0).
  - Normalize with the GroupNorm idiom: one tensor_scalar per segment with
    per-partition (mu, rstd) vector scalars, bf16 out.
  - PE transposes zln; MT computed as pm2[i,j] = zt^T E per batch half, so
    den = sum_i exp and num = diag(exp) are accumulating PE matmuls against
    a ones column (identity-masked product for the diagonal, on gpsimd).
  - Device outputs num/den [128, 16] f32; host does log(num/den + 1e-3) and
    the mean in float64.

ln_weight folds into W on the host; ln_bias cancels in the softmax.  No
max-subtraction needed: |logits| < ~70.
"""

import numpy as np

SEQ, B, L, ZD, CD = 2048, 64, 128, 128, 128
NCORES = 8
BC = B // NCORES  # 8
NSEG = 2 * BC  # 16 LN segments per core (interleaved pos/neg)
LN_EPS = 1e-5
SEM_STOP = 168  # min workable; exit sem sweep is ~65ns/sem

_cached = None


def _build_program():
    import concourse.bacc as bacc
    import concourse.tile as tile
    from concourse import bass as _bass
    from concourse import mybir

    # Fewer kernel semaphores -> shorter fixed entry/exit semaphore sweeps.
    orig_range = _bass.get_kernel_semaphore_range
    _bass.get_kernel_semaphore_range = lambda: range(
        orig_range().start, SEM_STOP
    )

    f32 = mybir.dt.float32
    bf16 = mybir.dt.bfloat16
    i32 = mybir.dt.int32
    AF = mybir.ActivationFunctionType
    ALU = mybir.AluOpType

    try:
        nc = bacc.Bacc(
            "TRN2",
            target_bir_lowering=False,
            debug=False,
            enable_asserts=True,
            num_devices=NCORES,
        )

        zg0_d = nc.dram_tensor("zg0", [128, 8 * ZD], bf16, kind="ExternalInput")
        zg1_d = nc.dram_tensor("zg1", [128, 8 * ZD], bf16, kind="ExternalInput")
        cgt_d = nc.dram_tensor("cgt", [128, BC * L], bf16, kind="ExternalInput")
        wpk_d = nc.dram_tensor("wpk", [128, 256], bf16, kind="ExternalInput")
        out_d = nc.dram_tensor("out", [128, NSEG], f32, kind="ExternalOutput")

        with tile.TileContext(nc) as tc:
            with (
                tc.tile_pool(name="singles", bufs=1) as singles,
                tc.tile_pool(name="szt", bufs=3) as szt,
                tc.tile_pool(name="sexp", bufs=2) as sexp,
                tc.tile_pool(name="ppe", bufs=2, space="PSUM") as ppe,
                tc.tile_pool(name="ppzt", bufs=2, space="PSUM") as ppzt,
                tc.tile_pool(name="ppmt", bufs=2, space="PSUM") as ppmt,
                tc.tile_pool(name="ppout", bufs=1, space="PSUM") as ppout,
            ):
                # ---- ACT table preload: sole set (exp_and_others) at t~0
                junk = singles.tile([128, 1], f32)
                nc.vector.memset(junk[:], 1.0)
                nc.scalar.activation(junk[:], junk[:], AF.Exp)

                # ---- input DMAs: zg quartered so stats start early ----
                zg = singles.tile([128, NSEG * ZD], bf16)
                for q in range(4):
                    src = (zg0_d, zg1_d)[q // 2]
                    half = (q % 2) * 512
                    nc.sync.dma_start(
                        out=zg[:, q * 512 : (q + 1) * 512],
                        in_=src.ap()[:, half : half + 512],
                    )
                wpk = singles.tile([128, 256], bf16)
                nc.sync.dma_start(wpk[:], wpk_d.ap())
                cgt = singles.tile([128, BC * L], bf16)
                nc.sync.dma_start(cgt[:], cgt_d.ap())
                wt = wpk[:, 0:128]
                identb = wpk[:, 128:256]

                # ---- c-side: E[z, b*L+j] = sum_c W'[z,c] c_t[j,b,c] ----
                e_sb = singles.tile([128, BC * L], bf16)
                for g in range(2):
                    pe = ppe.tile([128, 512], f32, tag="pe")
                    nc.tensor.matmul(
                        out=pe[:],
                        lhsT=wt,
                        rhs=cgt[:, g * 512 : (g + 1) * 512],
                        start=True,
                        stop=True,
                    )
                    nc.scalar.copy(e_sb[:, g * 512 : (g + 1) * 512], pe[:])

                # ---- LN stats: bn_stats per segment (HW: out = 6/part) ----
                st = singles.tile([128, NSEG, 6], f32)
                for s in range(NSEG):
                    nc.vector.bn_stats(
                        out=st[:, s, :], in_=zg[:, s * ZD : (s + 1) * ZD]
                    )

                # ---- gpsimd: even/odd recombine per half + Quake rsqrt ----
                # bn_stats gives (cnt, mean, cnt*var) of even and odd elems.
                # mu = (me+mo)/2;  128*(var+eps) = (sve+svo) + 32*d^2 + 128eps
                mu = singles.tile([128, NSEG], f32)
                dt_ = singles.tile([128, NSEG], f32)
                dd = singles.tile([128, NSEG], f32)
                vt = singles.tile([128, NSEG], f32)
                sv = singles.tile([128, NSEG], f32)
                vvx = singles.tile([128, NSEG], f32)

                def stv(h, k):
                    return st[:, 8 * h : 8 * h + 8, k : k + 1]

                for h in range(2):
                    hs = slice(8 * h, 8 * h + 8)
                    nc.gpsimd.tensor_tensor(
                        out=dt_[:, hs].unsqueeze(-1), in0=stv(h, 1),
                        in1=stv(h, 4), op=ALU.subtract,
                    )
                    nc.gpsimd.tensor_tensor(
                        out=dd[:, hs].unsqueeze(-1), in0=dt_[:, hs].unsqueeze(-1),
                        in1=dt_[:, hs].unsqueeze(-1), op=ALU.mult,
                    )
                    nc.gpsimd.tensor_scalar(
                        out=dd[:, hs], in0=dd[:, hs], scalar1=32.0,
                        scalar2=128.0 * LN_EPS, op0=ALU.mult, op1=ALU.add,
                    )
                    nc.gpsimd.tensor_tensor(
                        out=vt[:, hs].unsqueeze(-1), in0=stv(h, 2),
                        in1=stv(h, 5), op=ALU.add,
                    )
                    nc.gpsimd.tensor_tensor(
                        out=vvx[:, hs], in0=vt[:, hs], in1=dd[:, hs], op=ALU.add
                    )
                    nc.gpsimd.tensor_tensor(
                        out=sv[:, hs].unsqueeze(-1), in0=stv(h, 1),
                        in1=stv(h, 4), op=ALU.add,
                    )
                    nc.gpsimd.tensor_scalar(
                        out=mu[:, hs], in0=sv[:, hs], scalar1=0.5, scalar2=None,
                        op0=ALU.mult,
                    )
                # Quake rsqrt of vv = vvx/128 (the /128 and a final *sqrt(128)
                # fold into the magic constant and the Newton -0.5 scale).
                rstd = singles.tile([128, NSEG], f32)
                t1 = singles.tile([128, NSEG], f32)
                nc.vector.tensor_scalar(
                    out=rstd[:].bitcast(i32), in0=vvx[:].bitcast(i32),
                    scalar1=1, scalar2=None, op0=ALU.arith_shift_right,
                )
                nc.vector.tensor_scalar(
                    out=rstd[:].bitcast(i32), in0=rstd[:].bitcast(i32),
                    scalar1=-1, scalar2=0x5F3759DF + 0x01C00000,
                    op0=ALU.mult, op1=ALU.add,
                )
                for _ in range(2):
                    nc.gpsimd.tensor_tensor(
                        out=t1[:], in0=rstd[:], in1=rstd[:], op=ALU.mult
                    )
                    nc.gpsimd.tensor_tensor(
                        out=t1[:], in0=t1[:], in1=vvx[:], op=ALU.mult
                    )
                    nc.gpsimd.tensor_scalar(
                        out=t1[:], in0=t1[:], scalar1=-0.5 / 128.0, scalar2=1.5,
                        op0=ALU.mult, op1=ALU.add,
                    )
                    nc.gpsimd.tensor_tensor(
                        out=rstd[:], in0=rstd[:], in1=t1[:], op=ALU.mult
                    )

                # ---- normalize: one tensor_scalar per segment ----
                zln = singles.tile([128, NSEG * ZD], bf16)
                for s in range(NSEG):
                    nc.vector.tensor_scalar(
                        out=zln[:, s * ZD : (s + 1) * ZD],
                        in0=zg[:, s * ZD : (s + 1) * ZD],
                        scalar1=mu[:, s : s + 1],
                        scalar2=rstd[:, s : s + 1],
                        op0=ALU.subtract,
                        op1=ALU.mult,
                    )

                # ---- per pair p: transposes; MT in [i-part, j-free] layout;
                # den and num(diag) as accumulating PE matmuls vs ones col.
                outv = singles.tile([128, NSEG], f32)  # [num | den]
                onescol = singles.tile([128, 1], bf16)
                nc.vector.memset(onescol[:], 1.0)
                outp = ppout.tile([128, NSEG], f32, tag="outp")
                for p in range(4):
                    pzt = ppzt.tile([128, 512], bf16, tag="pzt")
                    for k in range(4):
                        s = 4 * p + k
                        nc.tensor.transpose(
                            out=pzt[:, k * 128 : (k + 1) * 128],
                            in_=zln[:, s * ZD : (s + 1) * ZD],
                            identity=identb,
                        )
                    zt = szt.tile([128, 512], bf16, tag="zt")
                    if p % 2 == 0:
                        nc.vector.tensor_copy(zt[:], pzt[:])
                    else:
                        nc.scalar.copy(zt[:], pzt[:])
                    # pm2 cols [(2k+h)*128]: batch 2p+k, half h (pos/neg i)
                    pm2 = ppmt.tile([128, 512], f32, tag="pm2")
                    for q in range(4):
                        b = 2 * p + q // 2
                        nc.tensor.matmul(
                            out=pm2[:, q * 128 : (q + 1) * 128],
                            lhsT=zt[:, q * 128 : (q + 1) * 128],
                            rhs=e_sb[:, b * L : (b + 1) * L],
                            start=True,
                            stop=True,
                        )
                    expm = sexp.tile([128, 512], bf16, tag="expm")
                    nc.scalar.activation(expm[:], pm2[:], AF.Exp)
                    for k in range(2):
                        b = 2 * p + k
                        # den[j,b] = sum_i expm[i, j]: accumulate both halves
                        nc.tensor.matmul(
                            out=outp[:, BC + b : BC + b + 1],
                            lhsT=expm[:, 2 * k * 128 : (2 * k + 1) * 128],
                            rhs=onescol[:],
                            start=True,
                            stop=False,
                        )
                        nc.tensor.matmul(
                            out=outp[:, BC + b : BC + b + 1],
                            lhsT=expm[:, (2 * k + 1) * 128 : (2 * k + 2) * 128],
                            rhs=onescol[:],
                            start=False,
                            stop=True,
                        )
                        # num[j,b] = expm[j,j] of the pos half (identity mask)
                        prod = szt.tile([128, 128], bf16, tag="prod")
                        nc.gpsimd.tensor_tensor(
                            out=prod[:],
                            in0=expm[:, 2 * k * 128 : (2 * k + 1) * 128],
                            in1=identb,
                            op=ALU.mult,
                        )
                        nc.tensor.matmul(
                            out=outp[:, b : b + 1],
                            lhsT=prod[:],
                            rhs=onescol[:],
                            start=True,
                            stop=True,
                        )
                nc.vector.tensor_copy(outv[:], outp[:])
                nc.sync.dma_start(out_d.ap(), outv[:])

        nc.compile()
        return nc
    finally:
        _bass.get_kernel_semaphore_range = orig_range


def _prep_in_maps(z, c, mask, neg_map, W, ln_weight):
    import ml_dtypes

    bf = ml_dtypes.bfloat16
    z = np.asarray(z, dtype=np.float32)
    c = np.asarray(c, dtype=np.float32)
    mask = np.asarray(mask).astype(np.int64)
    neg_map = np.asarray(neg_map).astype(np.int64)
    W = np.asarray(W, dtype=np.float32)
    ln_weight = np.asarray(ln_weight, dtype=np.float32)

    wt = (ln_weight[:, None] * W).T  # wt[c, z] = W'[z, c]
    wpk = np.ascontiguousarray(
        np.concatenate([wt, np.eye(128, dtype=np.float32)], axis=1)
    ).astype(bf)
    boff = np.arange(BC)[None, :]
    in_maps = []
    for i in range(NCORES):
        bsl = slice(i * BC, (i + 1) * BC)
        m = mask[:, bsl]  # [L, BC]
        n = neg_map[:, bsl]
        zb = z[:, bsl, :]
        cb = c[:, bsl, :]
        zpos = zb[m, boff, :]  # [L, BC, ZD]
        zneg = zb[n, boff, :]
        hit = (n[:, None, :] == m[None, :, :]).any(axis=1)  # [L, BC]
        zneg = np.where(hit[:, :, None], np.float32(0.0), zneg)
        zga = np.empty((L, NSEG, ZD), dtype=np.float32)
        zga[:, 0::2, :] = zpos
        zga[:, 1::2, :] = zneg
        zga16 = zga.astype(bf)
        zg0 = np.ascontiguousarray(zga16[:, 0:8, :].reshape(L, 8 * ZD))
        zg1 = np.ascontiguousarray(zga16[:, 8:16, :].reshape(L, 8 * ZD))
        cpos = cb[m, boff, :]  # [L(j), BC, CD]
        cgt = np.ascontiguousarray(
            cpos.transpose(2, 1, 0).reshape(CD, BC * L)
        ).astype(bf)
        in_maps.append({"zg0": zg0, "zg1": zg1, "cgt": cgt, "wpk": wpk})
    return in_maps


def _combine(results):
    total = np.float64(0.0)
    for r in results:
        o = np.asarray(r["out"], dtype=np.float64)
        num, den = o[:, 0:BC], o[:, BC : 2 * BC]
        total += np.log(num / den + 1e-3).sum()
    return np.float32(-(total / (L * B)))


def kernel(z, c, mask, neg_map, W, ln_weight, ln_bias):
    from concourse import bass_utils

    global _cached
    if _cached is None:
        _cached = _build_program()
    nc = _cached

    in_maps = _prep_in_maps(z, c, mask, neg_map, W, ln_weight)
    res = bass_utils.run_bass_kernel_spmd(
        nc, in_maps, core_ids=list(range(NCORES))
    )
    return _combine(res.results)
